# revision 6
# baseline (speedup 1.0000x reference)
"""ComplexAttention Trainium2 kernel — 8-core SPMD, head-parallel sharding.

Self-contained: kernel(**inputs) takes the FULL inputs (as in
reference.setup_inputs()) and returns the FULL [1,2048,512,2] output.

Per core c (heads 2c, 2c+1):
  - 4 projections (q, k, k' = [-ki;kr] for imag scores, v) as fp32r matmuls,
    contraction over 1024 interleaved real input channels, K=128-chunk PSUM
    accumulation. Outputs are channel-major [128ch, 2048seq] tiles.
  - per head, per sk-chunk (16x128): transposed score tiles
    sT_r/sT_i [sk=128, sq=2048] via single-shot K=64 fp32r matmuls
    (lhsT = kT/kTp chunk, rhs = qT; 1/sqrt(32) folded into Wq host-side).
  - softmax without max-subtraction (max|s| ~ 19 << 88):
    z = re^2+im^2 (custom DVE ops), |s| = exp(0.5*ln z), w = exp(|s|)
    (ACT, single resident natural_log_exp_and_others table set).
  - AV: outT[65, 2048] += v_chunk[128sk, 65].T @ w[128sk, 2048sq] with a
    ones column producing softmax denominators in row 64. fp32r K=128 accum.
  - normalize + (real,imag) interleave folded into the final PE transpose
    back to [sq, ch] layout; per-chunk [128, 128] stores.
"""
import os
import sys
import types

import numpy as np

for _p in ("/opt/trn_rl_repo", "/root/.axon_site/_ro/trn_rl_repo"):
    if _p not in sys.path and os.path.isdir(_p):
        sys.path.append(_p)

N_CORES = 8
S = 2048
C = 512
NH = 16
HD = 32
KCH = 1024          # interleaved real input channels = 2*C
N_SK = S // 128     # 16 sk chunks
SCALE = 1.0 / np.sqrt(np.float32(HD))

_PREPARED = None    # (nc, run_fn) cache across kernel() calls


# --------------------------------------------------------------------------
# framework tweaks (in-process only)
# --------------------------------------------------------------------------

def _patch_act_tables(keep="natural_log_exp_and_others"):
    """Make every ACT function resolve to the one table set that contains
    ln+exp+square+identity+copy, so the kernel never thrashes table loads."""
    import concourse.hw_specs as hw_specs
    import concourse.bacc as bacc_mod
    orig = hw_specs.get_activation_tables
    if getattr(bacc_mod.get_activation_tables, "_ca_patched", False):
        return

    def patched(arch):
        t = orig(arch)
        return {name: (funcs if name == keep else set())
                for name, funcs in t.items()}
    patched._ca_patched = True
    bacc_mod.get_activation_tables = patched


def _register_custom_ops():
    """Register SQ (x^2 from PSUM) and SQ_PLUS (x^2 + y) custom DVE ops."""
    import concourse.dve_ops as dmod
    from concourse.dve_ops import DveOp
    from concourse.dve_spec import Spec, Src0, Src1, sq, lower
    from concourse.dve_uop import DveOpSpec

    def reg(name, spec):
        if name in dmod._SUB_OPCODE_FOR_NAME:
            return next(o for o in dmod.OPS if o.name == name)
        row = dmod._CUSTOM_DVE_ROW_BASE + len(dmod.OPS)
        dmod._SUB_OPCODE_FOR_NAME[name] = row
        shas = {}
        from concourse.dve_spec import _has_src1
        for ver in ("v3", "v4"):
            tmp = DveOpSpec(name=name, opcode=row, uops=lower(spec, ver=ver),
                            rd1_en=_has_src1(spec))
            shas[ver] = tmp.sha(ver)
        op = DveOp(name, spec, subdim=False, uops_sha=shas)
        dmod.OPS.append(op)
        dmod.CUSTOM_DVE_SPECS[name] = op.spec
        return op

    sq_op = reg("CA_SQ", Spec(
        body=sq(Src0),
        reference=lambda in0, in1, s0, s1, imm2:
            (in0.astype(np.float32) ** 2).astype(np.float32)))
    sq_plus = reg("CA_SQ_PLUS", Spec(
        body=sq(Src0) + Src1,
        reference=lambda in0, in1, s0, s1, imm2:
            (in0.astype(np.float32) ** 2 + in1).astype(np.float32)))
    return sq_op, sq_plus


# --------------------------------------------------------------------------
# device program
# --------------------------------------------------------------------------

def _build_program():
    _patch_act_tables()
    SQ, SQ_PLUS = _register_custom_ops()

    import concourse.tile as tile
    from concourse import bacc, mybir

    f32 = mybir.dt.float32
    f32r = mybir.dt.float32r
    AF = mybir.ActivationFunctionType
    ALU = mybir.AluOpType

    nc = bacc.Bacc("TRN2", target_bir_lowering=False, debug=False,
                   num_devices=N_CORES)

    XT = {p: nc.dram_tensor(f"XT{p}", [KCH, S], f32r, kind="ExternalInput").ap()
          for p in ("q", "k", "v")}
    WB = {p: nc.dram_tensor(f"WB{p}", [128, KCH], f32r, kind="ExternalInput").ap()
          for p in ("q", "k", "kp", "v")}
    BIAS = nc.dram_tensor("BIAS", [128, 4], f32, kind="ExternalInput").ap()
    IDENT = nc.dram_tensor("IDENT", [128, 128], f32, kind="ExternalInput").ap()
    OUT = nc.dram_tensor("OUT", [S, 128], f32, kind="ExternalOutput").ap()

    with tile.TileContext(nc) as tc, \
         tc.tile_pool(name="persist", bufs=1) as persist, \
         tc.tile_pool(name="fin", bufs=1) as finp:
        with (
            tc.tile_pool(name="xstage", bufs=3) as xstage,
            tc.tile_pool(name="ps_proj", bufs=1, space="PSUM") as ps_proj,
        ):
            # const APs for activation bias immediates
            zeros1 = persist.tile([128, 1], f32)
            nc.gpsimd.memset(zeros1[:], 0.0)
            nc.const_aps.aps[(f32, 0.0)] = zeros1[:]
            eps1 = persist.tile([128, 1], f32)
            nc.gpsimd.memset(eps1[:], 1e-35)
            nc.const_aps.aps[(f32, 1e-35)] = eps1[:]

            ident = persist.tile([128, 128], f32)
            nc.sync.dma_start(ident[:], IDENT[:])
            bias = persist.tile([128, 4], f32)
            nc.sync.dma_start(bias[:], BIAS[:])

            wb = {}
            for p in ("q", "k", "kp", "v"):
                wb[p] = persist.tile([128, KCH], f32r, tag=f"wb{p}", name=f"wb{p}")
                nc.sync.dma_start(wb[p][:], WB[p][:])

            # ---------------- phase A: projections ----------------
            proj = {}
            bias_col = {"q": 0, "k": 1, "kp": 2, "v": 3}
            for p in ("v", "q", "k", "kp"):
                src = XT["k" if p == "kp" else p]
                psum_p = ps_proj.tile([128, S], f32, tag="proj")
                for kc in range(8):
                    xch = xstage.tile([128, S], f32r, tag="xch")
                    nc.sync.dma_start(xch[:], src[kc * 128:(kc + 1) * 128, :])
                    for g in range(4):
                        nc.tensor.matmul(
                            psum_p[:, g * 512:(g + 1) * 512],
                            wb[p][:, kc * 128:(kc + 1) * 128],
                            xch[:, g * 512:(g + 1) * 512],
                            start=(kc == 0), stop=(kc == 7))
                dt_out = f32 if p == "v" else f32r
                pt = persist.tile([128, S], dt_out, tag=f"proj{p}")
                nc.vector.tensor_scalar(
                    pt[:], psum_p[:], bias[:, bias_col[p]:bias_col[p] + 1],
                    None, ALU.add)
                proj[p] = pt

            # ---------------- phase A2: v chunks (transposed) + ones ------
            # v_ch[ck][p, hh*66 + j] = vT[hh*64+j, ck*128+p]; col hh*66+64 = 1
            v_ch = []
            for ck in range(N_SK):
                vc = persist.tile([128, 132], f32r, tag=f"vch{ck}")
                pt = ps_proj.tile([128, 128], f32, tag="vtr")
                nc.tensor.transpose(
                    pt[:], proj["v"][:, ck * 128:(ck + 1) * 128], ident[:])
                dst = vc[:].rearrange("p (h j) -> p h j", h=2)[:, :, 0:64]
                srcv = pt[:].rearrange("p (h j) -> p h j", h=2)
                nc.vector.tensor_copy(dst, srcv)
                ones_cols = vc[:].rearrange("p (h j) -> p h j", h=2)[:, :, 64:65]
                nc.gpsimd.memset(ones_cols.bitcast(f32), 1.0)
                v_ch.append(vc)

        # phase A pools closed; open phase B pools (reuse freed zones)
        with (
            tc.tile_pool(name="ps_sc", bufs=1, space="PSUM") as ps_sc,
            tc.tile_pool(name="ps_out", bufs=1, space="PSUM") as ps_out,
            tc.tile_pool(name="sbB", bufs=2) as sbB,
            tc.tile_pool(name="tq", bufs=2) as tq,
            tc.tile_pool(name="postp", bufs=2) as postp,
        ):
            final_tiles = [None] * N_SK

            for hh in range(2):
                qT = proj["q"][hh * 64:(hh + 1) * 64, :]
                kT = proj["k"][hh * 64:(hh + 1) * 64, :]
                kTp = proj["kp"][hh * 64:(hh + 1) * 64, :]
                outT = ps_out.tile([65, S], f32, tag="outT")

                for ck in range(N_SK):
                    ksl = slice(ck * 128, (ck + 1) * 128)
                    w_t = sbB.tile([128, S], f32r, tag="w")
                    z_t = sbB.tile([128, S], f32, tag="z")
                    for r2 in range(2):
                        a_t = ps_sc.tile([128, 1024], f32, tag="sa")
                        b_t = ps_sc.tile([128, 1024], f32, tag="sb")
                        for gg in range(2):
                            g0 = r2 * 1024 + gg * 512
                            nc.tensor.matmul(
                                a_t[:, gg * 512:(gg + 1) * 512],
                                kT[:, ksl], qT[:, g0:g0 + 512],
                                start=True, stop=True)
                            nc.tensor.matmul(
                                b_t[:, gg * 512:(gg + 1) * 512],
                                kTp[:, ksl], qT[:, g0:g0 + 512],
                                start=True, stop=True)
                        t_t = tq.tile([128, 1024], f32, tag="t")
                        nc.vector._custom_dve(SQ, out=t_t[:], in0=a_t[:])
                        nc.vector._custom_dve(
                            SQ_PLUS, out=z_t[:, r2 * 1024:(r2 + 1) * 1024],
                            in0=b_t[:], in1=t_t[:])
                    u_t = sbB.tile([128, S], f32, tag="u")
                    nc.scalar.activation(u_t[:], z_t[:], AF.Ln, bias=1e-35)
                    s_t = sbB.tile([128, S], f32, tag="s")
                    nc.scalar.activation(s_t[:], u_t[:], AF.Exp, scale=0.5)
                    nc.scalar.activation(w_t[:], s_t[:], AF.Exp)

                    # AV + denominator accumulation
                    vsl = v_ch[ck][:, hh * 66:hh * 66 + 65]
                    for g in range(4):
                        nc.tensor.matmul(
                            outT[0:65, g * 512:(g + 1) * 512],
                            vsl, w_t[:, g * 512:(g + 1) * 512],
                            start=(ck == 0), stop=(ck == N_SK - 1))

                # ---- postprocess head hh ----
                outS = postp.tile([65, S], f32, tag="outS")
                nc.vector.tensor_copy(outS[:], outT[0:65, :])
                denT = ps_sc.tile([128, 16], f32, tag="sa")
                for ck in range(N_SK):
                    nc.tensor.transpose(
                        denT[:, ck:ck + 1],
                        outS[64:65, ck * 128:(ck + 1) * 128],
                        ident[64:65, 64:65])
                recT = postp.tile([128, 16], f32, tag="recT")
                nc.vector.reciprocal_approx_fast(recT[:], denT[:])

                for ck in range(N_SK):
                    if hh == 0:
                        final_tiles[ck] = finp.tile(
                            [128, 128], f32, tag=f"fin{ck}", name=f"fin{ck}")
                    ft = final_tiles[ck]
                    pt = ps_sc.tile([128, 64], f32, tag="sb")
                    nc.tensor.transpose(
                        pt[:], outS[0:64, ck * 128:(ck + 1) * 128],
                        ident[0:64, 0:64])
                    src = pt[:].rearrange("p (t c) -> p t c", t=2)
                    dst = ft[:, hh * 64:(hh + 1) * 64].rearrange(
                        "p (c t) -> p t c", t=2)
                    nc.vector.tensor_scalar(
                        dst, src, recT[:, ck:ck + 1], None, ALU.mult)
                    if hh == 1:
                        nc.sync.dma_start(
                            OUT[ck * 128:(ck + 1) * 128, :], ft[:])

    nc.compile()
    return nc


# --------------------------------------------------------------------------
# host-side input prep
# --------------------------------------------------------------------------

def _prep_inputs(Q, V, K, Wq, bq, Wk, bk, Wv, bv):
    def xt(x):
        return np.ascontiguousarray(
            x.reshape(S, KCH).T.astype(np.float32, copy=False))

    shared = {
        "XTq": xt(np.asarray(Q)[0]),
        "XTk": xt(np.asarray(K)[0]),
        "XTv": xt(np.asarray(V)[0]),
        "IDENT": np.eye(128, dtype=np.float32),
    }

    # full-model lhsT blocks [1024 x 1024]; col = g*64 + type*32 + j,
    # o = g*32 + j; rows 2i (real in-ch), 2i+1 (imag in-ch)
    colsA = (np.arange(NH)[:, None] * 64 + np.arange(32)).ravel()
    colsB = colsA + 32
    osA = (np.arange(NH)[:, None] * 32 + np.arange(32)).ravel()

    def build(W, b, variant):
        W = np.asarray(W, dtype=np.float32)
        b = np.asarray(b, dtype=np.float32)
        Wr_t, Wi_t = W[:, :, 0].T, W[:, :, 1].T      # [i, o]
        lhsT = np.zeros((KCH, KCH), np.float32)
        biasv = np.zeros(KCH, np.float32)
        if variant == "std":
            lhsT[0::2][:, colsA] = Wr_t[:, osA]
            lhsT[1::2][:, colsA] = -Wi_t[:, osA]
            lhsT[0::2][:, colsB] = Wi_t[:, osA]
            lhsT[1::2][:, colsB] = Wr_t[:, osA]
            biasv[colsA] = b[osA, 0]
            biasv[colsB] = b[osA, 1]
        else:  # kp: rows [-ki ; kr]
            lhsT[0::2][:, colsA] = -Wi_t[:, osA]
            lhsT[1::2][:, colsA] = -Wr_t[:, osA]
            lhsT[0::2][:, colsB] = Wr_t[:, osA]
            lhsT[1::2][:, colsB] = -Wi_t[:, osA]
            biasv[colsA] = -b[osA, 1]
            biasv[colsB] = b[osA, 0]
        return lhsT, biasv

    lq, bq_v = build(Wq, bq, "std")
    lq *= SCALE
    bq_v = bq_v * SCALE
    lk, bk_v = build(Wk, bk, "std")
    lkp, bkp_v = build(Wk, bk, "kp")
    lv, bv_v = build(Wv, bv, "std")

    def chunked(blk):  # [1024, 128] -> [128, 8*128] chunk-major
        return np.ascontiguousarray(
            blk.reshape(8, 128, 128).transpose(1, 0, 2).reshape(128, KCH))

    in_maps = []
    for c in range(N_CORES):
        sl = slice(c * 128, (c + 1) * 128)
        m = dict(shared)
        m["WBq"] = chunked(lq[:, sl])
        m["WBk"] = chunked(lk[:, sl])
        m["WBkp"] = chunked(lkp[:, sl])
        m["WBv"] = chunked(lv[:, sl])
        m["BIAS"] = np.stack(
            [bq_v[sl], bk_v[sl], bkp_v[sl], bv_v[sl]], axis=1).astype(
                np.float32).copy()
        in_maps.append(m)
    return in_maps


# --------------------------------------------------------------------------
# entry point
# --------------------------------------------------------------------------

def _get_program():
    global _PREPARED
    if _PREPARED is None:
        _PREPARED = _build_program()
    return _PREPARED


def kernel(**inputs):
    from concourse.bass_utils import run_bass_kernel_spmd
    nc = _get_program()
    in_maps = _prep_inputs(**inputs)
    res = run_bass_kernel_spmd(nc, in_maps, list(range(N_CORES)), trace=False)
    parts = [res.results[c]["OUT"] for c in range(N_CORES)]
    full = np.concatenate(parts, axis=1)          # [2048, 1024]
    return full.reshape(1, S, C, 2).astype(np.float32, copy=False)


# revision 9
# speedup vs baseline: 1.2863x; 1.2863x over previous
"""ComplexAttention Trainium2 kernel — 8-core SPMD, head-parallel sharding.

Self-contained: kernel(**inputs) takes the FULL inputs (as in
reference.setup_inputs()) and returns the FULL [1,2048,512,2] output.

Per core c (heads 2c, 2c+1):
  - 4 projections (q, k, k' = [-ki;kr] for imag scores, v) as fp32r matmuls,
    contraction over 1024 interleaved real input channels, K=128-chunk PSUM
    accumulation. Outputs are channel-major [128ch, 2048seq] tiles.
  - per head, per sk-chunk (16x128): transposed score tiles
    sT_r/sT_i [sk=128, sq=2048] via single-shot K=64 fp32r matmuls
    (lhsT = kT/kTp chunk, rhs = qT; 1/sqrt(32) folded into Wq host-side).
  - softmax without max-subtraction (max|s| ~ 19 << 88):
    z = re^2+im^2 (custom DVE ops), |s| = exp(0.5*ln z), w = exp(|s|)
    (ACT, single resident natural_log_exp_and_others table set).
  - AV: outT[65, 2048] += v_chunk[128sk, 65].T @ w[128sk, 2048sq] with a
    ones column producing softmax denominators in row 64. fp32r K=128 accum.
  - normalize + (real,imag) interleave folded into the final PE transpose
    back to [sq, ch] layout; per-chunk [128, 128] stores.
"""
import os
import sys
import types

import numpy as np

for _p in ("/opt/trn_rl_repo", "/root/.axon_site/_ro/trn_rl_repo"):
    if _p not in sys.path and os.path.isdir(_p):
        sys.path.append(_p)

N_CORES = 8
S = 2048
C = 512
NH = 16
HD = 32
KCH = 1024          # interleaved real input channels = 2*C
N_SK = S // 128     # 16 sk chunks
SCALE = 1.0 / np.sqrt(np.float32(HD))

SQRT_MODE = os.environ.get("CA_SQRT_MODE", "sqrt")   # "sqrt" | "lnexp"
Z_FP16 = os.environ.get("CA_Z_FP16", "1") == "1"

_PREPARED = None    # compiled program cache across kernel() calls


# --------------------------------------------------------------------------
# framework tweaks (in-process only)
# --------------------------------------------------------------------------

def _patch_act_tables(keep=("sqrt_and_others", "exp_and_others",
                            "natural_log_exp_and_others")):
    """Restrict ACT table-set choice to a known set list so the inserter
    never picks a set that forces extra table loads."""
    import concourse.hw_specs as hw_specs
    import concourse.bacc as bacc_mod
    orig = hw_specs.get_activation_tables
    if getattr(bacc_mod.get_activation_tables, "_ca_patched", False):
        return

    def patched(arch):
        t = orig(arch)
        return {name: (funcs if name in keep else set())
                for name, funcs in t.items()}
    patched._ca_patched = True
    bacc_mod.get_activation_tables = patched


def _register_custom_ops():
    """Register SQ (x^2 from PSUM) and SQ_PLUS (x^2 + y) custom DVE ops."""
    import concourse.dve_ops as dmod
    from concourse.dve_ops import DveOp
    from concourse.dve_spec import Spec, Src0, Src1, sq, lower
    from concourse.dve_uop import DveOpSpec

    def reg(name, spec):
        if name in dmod._SUB_OPCODE_FOR_NAME:
            return next(o for o in dmod.OPS if o.name == name)
        row = dmod._CUSTOM_DVE_ROW_BASE + len(dmod.OPS)
        dmod._SUB_OPCODE_FOR_NAME[name] = row
        shas = {}
        from concourse.dve_spec import _has_src1
        for ver in ("v3", "v4"):
            tmp = DveOpSpec(name=name, opcode=row, uops=lower(spec, ver=ver),
                            rd1_en=_has_src1(spec))
            shas[ver] = tmp.sha(ver)
        op = DveOp(name, spec, subdim=False, uops_sha=shas)
        dmod.OPS.append(op)
        dmod.CUSTOM_DVE_SPECS[name] = op.spec
        return op

    sq_op = reg("CA_SQ", Spec(
        body=sq(Src0),
        reference=lambda in0, in1, s0, s1, imm2:
            (in0.astype(np.float32) ** 2).astype(np.float32)))
    sq_plus = reg("CA_SQ_PLUS", Spec(
        body=sq(Src0) + Src1,
        reference=lambda in0, in1, s0, s1, imm2:
            (in0.astype(np.float32) ** 2 + in1).astype(np.float32)))
    return sq_op, sq_plus


# --------------------------------------------------------------------------
# device program
# --------------------------------------------------------------------------

def _build_program():
    keep = (("sqrt_and_others", "exp_and_others") if SQRT_MODE == "sqrt"
            else ("natural_log_exp_and_others",))
    _patch_act_tables(keep)
    SQ, SQ_PLUS = _register_custom_ops()

    import concourse.tile as tile
    from concourse import bacc, mybir

    f32 = mybir.dt.float32
    f16 = mybir.dt.float16
    f32r = mybir.dt.float32r
    AF = mybir.ActivationFunctionType
    ALU = mybir.AluOpType
    zdt = f16 if Z_FP16 else f32

    nc = bacc.Bacc("TRN2", target_bir_lowering=False, debug=False,
                   num_devices=N_CORES)

    XT = {p: nc.dram_tensor(f"XT{p}", [KCH, S], f32r, kind="ExternalInput").ap()
          for p in ("q", "k", "v")}
    WB = {p: nc.dram_tensor(f"WB{p}", [128, KCH], f32r, kind="ExternalInput").ap()
          for p in ("q", "k", "v")}
    BIAS = nc.dram_tensor("BIAS", [128, 4], f32, kind="ExternalInput").ap()
    IDENT = nc.dram_tensor("IDENT", [128, 128], f32, kind="ExternalInput").ap()
    PERM = nc.dram_tensor("PERM", [128, 128], f32r, kind="ExternalInput").ap()
    OUT = nc.dram_tensor("OUT", [S, 128], f32, kind="ExternalOutput").ap()

    with tile.TileContext(nc) as tc, \
         tc.tile_pool(name="persist", bufs=1) as persist, \
         tc.tile_pool(name="fin", bufs=1) as finp:
        with (
            tc.tile_pool(name="xstage", bufs=4) as xstage,
            tc.tile_pool(name="ps_proj", bufs=2, space="PSUM") as ps_proj,
        ):
            zeros1 = persist.tile([128, 1], f32)
            nc.gpsimd.memset(zeros1[:], 0.0)
            nc.const_aps.aps[(f32, 0.0)] = zeros1[:]
            eps1 = persist.tile([128, 1], f32)
            nc.gpsimd.memset(eps1[:], 1e-35)
            nc.const_aps.aps[(f32, 1e-35)] = eps1[:]

            ident = persist.tile([128, 128], f32)
            nc.sync.dma_start(ident[:], IDENT[:])
            bias = persist.tile([128, 4], f32)
            nc.sync.dma_start(bias[:], BIAS[:])
            permT = persist.tile([128, 128], f32r)
            nc.sync.dma_start(permT[:], PERM[:])

            wb = {}
            for p in ("q", "k", "v"):
                wb[p] = persist.tile([128, KCH], f32r, tag=f"wb{p}",
                                     name=f"wb{p}")
                nc.sync.dma_start(wb[p][:], WB[p][:])

            # ---- phase A: projections, order v -> q -> k ----
            proj = {}
            bias_col = {"q": 0, "k": 1, "v": 3}
            for p in ("v", "q", "k"):
                psum_p = ps_proj.tile([128, S], f32, tag="proj",
                                      name=f"psum_{p}")
                for kc in range(8):
                    xch = xstage.tile([128, S], f32r, tag="xch")
                    nc.sync.dma_start(xch[:], XT[p][kc * 128:(kc + 1) * 128, :])
                    for g in range(4):
                        nc.tensor.matmul(
                            psum_p[:, g * 512:(g + 1) * 512],
                            wb[p][:, kc * 128:(kc + 1) * 128],
                            xch[:, g * 512:(g + 1) * 512],
                            start=(kc == 0), stop=(kc == 7))
                dt_out = f32 if p == "v" else f32r
                pt = persist.tile([128, S], dt_out, tag=f"proj{p}",
                                  name=f"proj{p}")
                nc.scalar.activation(pt[:], psum_p[:], AF.Identity,
                                     bias=bias[:, bias_col[p]:bias_col[p] + 1])
                proj[p] = pt

                if p == "v":
                    # v chunks (transposed) + ones columns, reuse proj slots
                    v_ch = []
                    for ck in range(N_SK):
                        vc = persist.tile([128, 132], f32r, tag=f"vch{ck}",
                                          name=f"vch{ck}")
                        ptr = ps_proj.tile([128, 128], f32, tag="proj",
                                           name=f"vtr{ck}")
                        nc.tensor.transpose(
                            ptr[:], pt[:, ck * 128:(ck + 1) * 128], ident[:])
                        dst = vc[:].rearrange(
                            "p (h j) -> p h j", h=2)[:, :, 0:64]
                        srcv = ptr[:].rearrange("p (h j) -> p h j", h=2)
                        nc.vector.tensor_copy(dst, srcv)
                        ones_cols = vc[:].rearrange(
                            "p (h j) -> p h j", h=2)[:, :, 64:65]
                        nc.gpsimd.memset(ones_cols.bitcast(f32), 1.0)
                        v_ch.append(vc)

            # kTp = Perm @ kT (signed block-swap of the biased kT rows):
            # row hh*64+j (j<32) = -kT[hh*64+32+j]; row hh*64+32+j = kT[hh*64+j]
            kTp_t = persist.tile([128, S], f32r, tag="projkp", name="projkp")
            for g in range(4):
                pp = ps_proj.tile([128, 512], f32, tag="proj",
                                  name=f"kppsum{g}")
                nc.tensor.matmul(pp[:], permT[:],
                                 proj["k"][:, g * 512:(g + 1) * 512],
                                 start=True, stop=True)
                nc.scalar.activation(kTp_t[:, g * 512:(g + 1) * 512], pp[:],
                                     AF.Identity)
            proj["kp"] = kTp_t

        # ---- phase B ----
        with (
            tc.tile_pool(name="ps_sc", bufs=2, space="PSUM") as ps_sc,
            tc.tile_pool(name="ps_out", bufs=1, space="PSUM") as ps_out,
            tc.tile_pool(name="zpool", bufs=6) as zpool,
            tc.tile_pool(name="spool", bufs=4) as spool,
            tc.tile_pool(name="wpool", bufs=4) as wpool,
            tc.tile_pool(name="tq", bufs=2) as tq,
            tc.tile_pool(name="postp", bufs=2) as postp,
        ):
            final_tiles = [None] * N_SK
            NB = 4                       # cks per ACT table batch

            for hh in range(2):
                qT = proj["q"][hh * 64:(hh + 1) * 64, :]
                kT = proj["k"][hh * 64:(hh + 1) * 64, :]
                kTp = proj["kp"][hh * 64:(hh + 1) * 64, :]
                outT = ps_out.tile([65, S], f32, tag="outT", name=f"outT{hh}")

                for b in range(N_SK // NB):
                    cks = range(b * NB, (b + 1) * NB)
                    z_ts, s_ts, w_ts = {}, {}, {}
                    for ck in cks:
                        ksl = slice(ck * 128, (ck + 1) * 128)
                        z_t = zpool.tile([128, S], zdt, tag="z",
                                         name=f"z{hh}_{ck}")
                        z_ts[ck] = z_t
                        for r2 in range(2):
                            a_t = ps_sc.tile([128, 1024], f32, tag="sc",
                                             name=f"sa{hh}_{ck}_{r2}")
                            b_t = ps_sc.tile([128, 1024], f32, tag="sc",
                                             name=f"sb{hh}_{ck}_{r2}")
                            for gg in range(2):
                                g0 = r2 * 1024 + gg * 512
                                nc.tensor.matmul(
                                    a_t[:, gg * 512:(gg + 1) * 512],
                                    kT[:, ksl], qT[:, g0:g0 + 512],
                                    start=True, stop=True)
                            for gg in range(2):
                                g0 = r2 * 1024 + gg * 512
                                nc.tensor.matmul(
                                    b_t[:, gg * 512:(gg + 1) * 512],
                                    kTp[:, ksl], qT[:, g0:g0 + 512],
                                    start=True, stop=True)
                            t_t = tq.tile([128, 1024], f32, tag="t")
                            nc.vector._custom_dve(SQ, out=t_t[:], in0=a_t[:])
                            nc.vector._custom_dve(
                                SQ_PLUS,
                                out=z_t[:, r2 * 1024:(r2 + 1) * 1024],
                                in0=b_t[:], in1=t_t[:])
                    for ck in cks:
                        s_t = spool.tile([128, S], f32, tag="s",
                                         name=f"s{hh}_{ck}")
                        s_ts[ck] = s_t
                        if SQRT_MODE == "sqrt":
                            nc.scalar.activation(s_t[:], z_ts[ck][:], AF.Sqrt)
                        else:
                            nc.scalar.activation(s_t[:], z_ts[ck][:], AF.Ln,
                                                 bias=1e-35)
                    for ck in cks:
                        w_t = wpool.tile([128, S], f32r, tag="w",
                                         name=f"w{hh}_{ck}")
                        w_ts[ck] = w_t
                        if SQRT_MODE == "sqrt":
                            nc.scalar.activation(w_t[:], s_ts[ck][:], AF.Exp)
                        else:
                            u2 = spool.tile([128, S], f32, tag="s",
                                            name=f"u2{hh}_{ck}")
                            nc.scalar.activation(u2[:], s_ts[ck][:], AF.Exp,
                                                 scale=0.5)
                            nc.scalar.activation(w_t[:], u2[:], AF.Exp)
                    for ck in cks:
                        vsl = v_ch[ck][:, hh * 66:hh * 66 + 65]
                        for g in range(4):
                            nc.tensor.matmul(
                                outT[0:65, g * 512:(g + 1) * 512],
                                vsl, w_ts[ck][:, g * 512:(g + 1) * 512],
                                start=(ck == 0), stop=(ck == N_SK - 1))

                # ---- postprocess head hh ----
                outS = postp.tile([65, S], f32, tag="outS", name=f"outS{hh}")
                nc.scalar.copy(outS[:], outT[0:65, :])
                denT = ps_sc.tile([128, 16], f32, tag="sc", name=f"denT{hh}")
                for ck in range(N_SK):
                    nc.tensor.transpose(
                        denT[:, ck:ck + 1],
                        outS[64:65, ck * 128:(ck + 1) * 128],
                        ident[64:65, 64:65])
                recT = postp.tile([128, 16], f32, tag="recT", name=f"recT{hh}")
                nc.vector.reciprocal_approx_fast(recT[:], denT[:])

                for ck in range(N_SK):
                    if hh == 0:
                        final_tiles[ck] = finp.tile(
                            [128, 128], f32, tag=f"fin{ck}", name=f"fin{ck}")
                    ft = final_tiles[ck]
                    ptf = ps_sc.tile([128, 64], f32, tag="sc",
                                     name=f"ftr{hh}_{ck}")
                    nc.tensor.transpose(
                        ptf[:], outS[0:64, ck * 128:(ck + 1) * 128],
                        ident[0:64, 0:64])
                    src = ptf[:].rearrange("p (t c) -> p t c", t=2)
                    dst = ft[:, hh * 64:(hh + 1) * 64].rearrange(
                        "p (c t) -> p t c", t=2)
                    nc.vector.tensor_scalar(
                        dst, src, recT[:, ck:ck + 1], None, ALU.mult)
                    if hh == 1:
                        nc.sync.dma_start(
                            OUT[ck * 128:(ck + 1) * 128, :], ft[:])

    nc.compile()
    return nc


# --------------------------------------------------------------------------
# host-side input prep
# --------------------------------------------------------------------------

def _prep_inputs(Q, V, K, Wq, bq, Wk, bk, Wv, bv):
    def xt(x):
        return np.ascontiguousarray(
            x.reshape(S, KCH).T.astype(np.float32, copy=False))

    perm = np.zeros((128, 128), np.float32)
    for _hh in range(2):
        r0 = _hh * 64
        for j in range(32):
            perm[r0 + j, r0 + 32 + j] = -1.0
            perm[r0 + 32 + j, r0 + j] = 1.0
    shared = {
        "XTq": xt(np.asarray(Q)[0]),
        "XTk": xt(np.asarray(K)[0]),
        "XTv": xt(np.asarray(V)[0]),
        "IDENT": np.eye(128, dtype=np.float32),
        "PERM": np.ascontiguousarray(perm.T),
    }

    # full-model lhsT blocks [1024 x 1024]; col = g*64 + type*32 + j,
    # o = g*32 + j; rows 2i (real in-ch), 2i+1 (imag in-ch)
    colsA = (np.arange(NH)[:, None] * 64 + np.arange(32)).ravel()
    colsB = colsA + 32
    osA = (np.arange(NH)[:, None] * 32 + np.arange(32)).ravel()

    def build(W, b, variant):
        W = np.asarray(W, dtype=np.float32)
        b = np.asarray(b, dtype=np.float32)
        Wr_t, Wi_t = W[:, :, 0].T, W[:, :, 1].T      # [i, o]
        lhsT = np.zeros((KCH, KCH), np.float32)
        biasv = np.zeros(KCH, np.float32)
        if variant == "std":
            lhsT[0::2][:, colsA] = Wr_t[:, osA]
            lhsT[1::2][:, colsA] = -Wi_t[:, osA]
            lhsT[0::2][:, colsB] = Wi_t[:, osA]
            lhsT[1::2][:, colsB] = Wr_t[:, osA]
            biasv[colsA] = b[osA, 0]
            biasv[colsB] = b[osA, 1]
        else:  # kp: rows [-ki ; kr]
            lhsT[0::2][:, colsA] = -Wi_t[:, osA]
            lhsT[1::2][:, colsA] = -Wr_t[:, osA]
            lhsT[0::2][:, colsB] = Wr_t[:, osA]
            lhsT[1::2][:, colsB] = -Wi_t[:, osA]
            biasv[colsA] = -b[osA, 1]
            biasv[colsB] = b[osA, 0]
        return lhsT, biasv

    lq, bq_v = build(Wq, bq, "std")
    lq *= SCALE
    bq_v = bq_v * SCALE
    lk, bk_v = build(Wk, bk, "std")
    lkp, bkp_v = build(Wk, bk, "kp")
    lv, bv_v = build(Wv, bv, "std")

    def chunked(blk):  # [1024, 128] -> [128, 8*128] chunk-major
        return np.ascontiguousarray(
            blk.reshape(8, 128, 128).transpose(1, 0, 2).reshape(128, KCH))

    in_maps = []
    for c in range(N_CORES):
        sl = slice(c * 128, (c + 1) * 128)
        m = dict(shared)
        m["WBq"] = chunked(lq[:, sl])
        m["WBk"] = chunked(lk[:, sl])
        m["WBv"] = chunked(lv[:, sl])
        m["BIAS"] = np.stack(
            [bq_v[sl], bk_v[sl], bkp_v[sl], bv_v[sl]], axis=1).astype(
                np.float32).copy()
        in_maps.append(m)
    return in_maps


# --------------------------------------------------------------------------
# entry point
# --------------------------------------------------------------------------

def _get_program():
    global _PREPARED
    if _PREPARED is None:
        _PREPARED = _build_program()
    return _PREPARED


def kernel(**inputs):
    from concourse.bass_utils import run_bass_kernel_spmd
    nc = _get_program()
    in_maps = _prep_inputs(**inputs)
    res = run_bass_kernel_spmd(nc, in_maps, list(range(N_CORES)), trace=False)
    parts = [res.results[c]["OUT"] for c in range(N_CORES)]
    full = np.concatenate(parts, axis=1)          # [2048, 1024]
    return full.reshape(1, S, C, 2).astype(np.float32, copy=False)


# revision 10
# speedup vs baseline: 1.4536x; 1.1301x over previous
"""ComplexAttention Trainium2 kernel — 8-core SPMD, head-parallel sharding.

Self-contained: kernel(**inputs) takes the FULL inputs (as in
reference.setup_inputs()) and returns the FULL [1,2048,512,2] output.

Per core c (heads 2c, 2c+1):
  - 4 projections (q, k, k' = [-ki;kr] for imag scores, v) as fp32r matmuls,
    contraction over 1024 interleaved real input channels, K=128-chunk PSUM
    accumulation. Outputs are channel-major [128ch, 2048seq] tiles.
  - per head, per sk-chunk (16x128): transposed score tiles
    sT_r/sT_i [sk=128, sq=2048] via single-shot K=64 fp32r matmuls
    (lhsT = kT/kTp chunk, rhs = qT; 1/sqrt(32) folded into Wq host-side).
  - softmax without max-subtraction (max|s| ~ 19 << 88):
    z = re^2+im^2 (custom DVE ops), |s| = exp(0.5*ln z), w = exp(|s|)
    (ACT, single resident natural_log_exp_and_others table set).
  - AV: outT[65, 2048] += v_chunk[128sk, 65].T @ w[128sk, 2048sq] with a
    ones column producing softmax denominators in row 64. fp32r K=128 accum.
  - normalize + (real,imag) interleave folded into the final PE transpose
    back to [sq, ch] layout; per-chunk [128, 128] stores.
"""
import os
import sys
import types

import numpy as np

for _p in ("/opt/trn_rl_repo", "/root/.axon_site/_ro/trn_rl_repo"):
    if _p not in sys.path and os.path.isdir(_p):
        sys.path.append(_p)

N_CORES = 8
S = 2048
C = 512
NH = 16
HD = 32
KCH = 1024          # interleaved real input channels = 2*C
N_SK = S // 128     # 16 sk chunks
SCALE = 1.0 / np.sqrt(np.float32(HD))

SQRT_MODE = os.environ.get("CA_SQRT_MODE", "sqrt")   # "sqrt" | "lnexp"
W_SHIFT = 14.0      # w = exp(|s| - W_SHIFT); constant shift cancels in softmax
Z_FP16 = os.environ.get("CA_Z_FP16", "1") == "1"

_PREPARED = None    # compiled program cache across kernel() calls


# --------------------------------------------------------------------------
# framework tweaks (in-process only)
# --------------------------------------------------------------------------

def _patch_act_tables(keep=("sqrt_and_others", "exp_and_others",
                            "natural_log_exp_and_others")):
    """Restrict ACT table-set choice to a known set list so the inserter
    never picks a set that forces extra table loads."""
    import concourse.hw_specs as hw_specs
    import concourse.bacc as bacc_mod
    orig = hw_specs.get_activation_tables
    if getattr(bacc_mod.get_activation_tables, "_ca_patched", False):
        return

    def patched(arch):
        t = orig(arch)
        return {name: (funcs if name in keep else set())
                for name, funcs in t.items()}
    patched._ca_patched = True
    bacc_mod.get_activation_tables = patched


def _register_custom_ops():
    """Register SQ (x^2 from PSUM) and SQ_PLUS (x^2 + y) custom DVE ops."""
    import concourse.dve_ops as dmod
    from concourse.dve_ops import DveOp
    from concourse.dve_spec import Spec, Src0, Src1, sq, lower
    from concourse.dve_uop import DveOpSpec

    def reg(name, spec):
        if name in dmod._SUB_OPCODE_FOR_NAME:
            return next(o for o in dmod.OPS if o.name == name)
        row = dmod._CUSTOM_DVE_ROW_BASE + len(dmod.OPS)
        dmod._SUB_OPCODE_FOR_NAME[name] = row
        shas = {}
        from concourse.dve_spec import _has_src1
        for ver in ("v3", "v4"):
            tmp = DveOpSpec(name=name, opcode=row, uops=lower(spec, ver=ver),
                            rd1_en=_has_src1(spec))
            shas[ver] = tmp.sha(ver)
        op = DveOp(name, spec, subdim=False, uops_sha=shas)
        dmod.OPS.append(op)
        dmod.CUSTOM_DVE_SPECS[name] = op.spec
        return op

    sq_op = reg("CA_SQ", Spec(
        body=sq(Src0),
        reference=lambda in0, in1, s0, s1, imm2:
            (in0.astype(np.float32) ** 2).astype(np.float32)))
    sq_plus = reg("CA_SQ_PLUS", Spec(
        body=sq(Src0) + Src1,
        reference=lambda in0, in1, s0, s1, imm2:
            (in0.astype(np.float32) ** 2 + in1).astype(np.float32)))
    return sq_op, sq_plus


# --------------------------------------------------------------------------
# device program
# --------------------------------------------------------------------------

def _build_program():
    keep = (("sqrt_and_others", "exp_and_others") if SQRT_MODE == "sqrt"
            else ("natural_log_exp_and_others",))
    _patch_act_tables(keep)
    SQ, SQ_PLUS = _register_custom_ops()

    import concourse.tile as tile
    from concourse import bacc, mybir

    f32 = mybir.dt.float32
    f16 = mybir.dt.float16
    f32r = mybir.dt.float32r
    AF = mybir.ActivationFunctionType
    ALU = mybir.AluOpType
    zdt = f16 if Z_FP16 else f32

    nc = bacc.Bacc("TRN2", target_bir_lowering=False, debug=False,
                   num_devices=N_CORES)

    XT = {p: nc.dram_tensor(f"XT{p}", [KCH, S], f16, kind="ExternalInput").ap()
          for p in ("q", "k", "v")}
    WB = {p: nc.dram_tensor(f"WB{p}", [128, KCH], f16, kind="ExternalInput").ap()
          for p in ("q", "k", "v")}
    BIAS = nc.dram_tensor("BIAS", [128, 4], f32, kind="ExternalInput").ap()
    IDENT = nc.dram_tensor("IDENT", [128, 128], f32, kind="ExternalInput").ap()
    PERM = nc.dram_tensor("PERM", [128, 128], f16, kind="ExternalInput").ap()
    OUT = nc.dram_tensor("OUT", [S, 128], f32, kind="ExternalOutput").ap()

    with tile.TileContext(nc) as tc, \
         tc.tile_pool(name="persist", bufs=1) as persist, \
         tc.tile_pool(name="fin", bufs=1) as finp:
        with (
            tc.tile_pool(name="xstage", bufs=4) as xstage,
            tc.tile_pool(name="ps_proj", bufs=2, space="PSUM") as ps_proj,
        ):
            zeros1 = persist.tile([128, 1], f32)
            nc.gpsimd.memset(zeros1[:], 0.0)
            nc.const_aps.aps[(f32, 0.0)] = zeros1[:]
            eps1 = persist.tile([128, 1], f32)
            nc.gpsimd.memset(eps1[:], 1e-35)
            nc.const_aps.aps[(f32, 1e-35)] = eps1[:]
            wsh = persist.tile([128, 1], f32)
            nc.gpsimd.memset(wsh[:], -W_SHIFT)
            nc.const_aps.aps[(f32, -W_SHIFT)] = wsh[:]

            ident = persist.tile([128, 128], f32)
            nc.sync.dma_start(ident[:], IDENT[:])
            bias = persist.tile([128, 4], f32)
            nc.sync.dma_start(bias[:], BIAS[:])
            permT = persist.tile([128, 128], f16)
            nc.sync.dma_start(permT[:], PERM[:])

            wb = {}
            for p in ("q", "k", "v"):
                wb[p] = persist.tile([128, KCH], f16, tag=f"wb{p}",
                                     name=f"wb{p}")
                nc.sync.dma_start(wb[p][:], WB[p][:])

            # ---- phase A: projections, order v -> q -> k ----
            proj = {}
            bias_col = {"q": 0, "k": 1, "v": 3}
            for p in ("v", "q", "k"):
                psum_p = ps_proj.tile([128, S], f32, tag="proj",
                                      name=f"psum_{p}")
                for kc in range(8):
                    xch = xstage.tile([128, S], f16, tag="xch")
                    nc.sync.dma_start(xch[:], XT[p][kc * 128:(kc + 1) * 128, :])
                    for g in range(4):
                        nc.tensor.matmul(
                            psum_p[:, g * 512:(g + 1) * 512],
                            wb[p][:, kc * 128:(kc + 1) * 128],
                            xch[:, g * 512:(g + 1) * 512],
                            start=(kc == 0), stop=(kc == 7))
                dt_out = f32 if p == "v" else f16
                pt = persist.tile([128, S], dt_out, tag=f"proj{p}",
                                  name=f"proj{p}")
                nc.scalar.activation(pt[:], psum_p[:], AF.Identity,
                                     bias=bias[:, bias_col[p]:bias_col[p] + 1])
                proj[p] = pt

                if p == "v":
                    # v chunks (transposed) + ones columns, reuse proj slots
                    v_ch = []
                    for ck in range(N_SK):
                        vc = persist.tile([128, 132], f16, tag=f"vch{ck}",
                                          name=f"vch{ck}")
                        ptr = ps_proj.tile([128, 128], f32, tag="proj",
                                           name=f"vtr{ck}")
                        nc.tensor.transpose(
                            ptr[:], pt[:, ck * 128:(ck + 1) * 128], ident[:])
                        dst = vc[:].rearrange(
                            "p (h j) -> p h j", h=2)[:, :, 0:64]
                        srcv = ptr[:].rearrange("p (h j) -> p h j", h=2)
                        nc.vector.tensor_copy(dst, srcv)
                        ones_cols = vc[:].rearrange(
                            "p (h j) -> p h j", h=2)[:, :, 64:65]
                        nc.gpsimd.memset(ones_cols, 1.0)
                        v_ch.append(vc)

            # kTp = Perm @ kT (signed block-swap of the biased kT rows):
            # row hh*64+j (j<32) = -kT[hh*64+32+j]; row hh*64+32+j = kT[hh*64+j]
            kTp_t = persist.tile([128, S], f16, tag="projkp", name="projkp")
            for g in range(4):
                pp = ps_proj.tile([128, 512], f32, tag="proj",
                                  name=f"kppsum{g}")
                nc.tensor.matmul(pp[:], permT[:],
                                 proj["k"][:, g * 512:(g + 1) * 512],
                                 start=True, stop=True)
                nc.scalar.activation(kTp_t[:, g * 512:(g + 1) * 512], pp[:],
                                     AF.Identity)
            proj["kp"] = kTp_t

        # ---- phase B ----
        with (
            tc.tile_pool(name="ps_sc", bufs=2, space="PSUM") as ps_sc,
            tc.tile_pool(name="ps_out", bufs=1, space="PSUM") as ps_out,
            tc.tile_pool(name="zpool", bufs=6) as zpool,
            tc.tile_pool(name="spool", bufs=5) as spool,
            tc.tile_pool(name="wpool", bufs=6) as wpool,
            tc.tile_pool(name="tq", bufs=2) as tq,
            tc.tile_pool(name="postp", bufs=2) as postp,
        ):
            final_tiles = [None] * N_SK
            NB = 4                       # cks per ACT table batch

            for hh in range(2):
                qT = proj["q"][hh * 64:(hh + 1) * 64, :]
                kT = proj["k"][hh * 64:(hh + 1) * 64, :]
                kTp = proj["kp"][hh * 64:(hh + 1) * 64, :]
                outT = ps_out.tile([65, S], f32, tag="outT", name=f"outT{hh}")

                for b in range(N_SK // NB):
                    cks = range(b * NB, (b + 1) * NB)
                    z_ts, s_ts, w_ts = {}, {}, {}
                    for ck in cks:
                        ksl = slice(ck * 128, (ck + 1) * 128)
                        z_t = zpool.tile([128, S], zdt, tag="z",
                                         name=f"z{hh}_{ck}")
                        z_ts[ck] = z_t
                        for r2 in range(2):
                            a_t = ps_sc.tile([128, 1024], f32, tag="sc",
                                             name=f"sa{hh}_{ck}_{r2}")
                            b_t = ps_sc.tile([128, 1024], f32, tag="sc",
                                             name=f"sb{hh}_{ck}_{r2}")
                            for gg in range(2):
                                g0 = r2 * 1024 + gg * 512
                                nc.tensor.matmul(
                                    a_t[:, gg * 512:(gg + 1) * 512],
                                    kT[:, ksl], qT[:, g0:g0 + 512],
                                    start=True, stop=True)
                            for gg in range(2):
                                g0 = r2 * 1024 + gg * 512
                                nc.tensor.matmul(
                                    b_t[:, gg * 512:(gg + 1) * 512],
                                    kTp[:, ksl], qT[:, g0:g0 + 512],
                                    start=True, stop=True)
                            t_t = tq.tile([128, 1024], f32, tag="t")
                            nc.vector._custom_dve(SQ, out=t_t[:], in0=a_t[:])
                            nc.vector._custom_dve(
                                SQ_PLUS,
                                out=z_t[:, r2 * 1024:(r2 + 1) * 1024],
                                in0=b_t[:], in1=t_t[:])
                    for ck in cks:
                        s_t = spool.tile([128, S], f32, tag="s",
                                         name=f"s{hh}_{ck}")
                        s_ts[ck] = s_t
                        if SQRT_MODE == "sqrt":
                            nc.scalar.activation(s_t[:], z_ts[ck][:], AF.Sqrt)
                        else:
                            nc.scalar.activation(s_t[:], z_ts[ck][:], AF.Ln,
                                                 bias=1e-35)
                    for ck in cks:
                        w_t = wpool.tile([128, S], f16, tag="w",
                                         name=f"w{hh}_{ck}")
                        w_ts[ck] = w_t
                        if SQRT_MODE == "sqrt":
                            nc.scalar.activation(w_t[:], s_ts[ck][:], AF.Exp,
                                                 bias=-W_SHIFT)
                        else:
                            u2 = spool.tile([128, S], f32, tag="s",
                                            name=f"u2{hh}_{ck}")
                            nc.scalar.activation(u2[:], s_ts[ck][:], AF.Exp,
                                                 scale=0.5)
                            nc.scalar.activation(w_t[:], u2[:], AF.Exp,
                                                 bias=-W_SHIFT)
                    for ck in cks:
                        vsl = v_ch[ck][:, hh * 66:hh * 66 + 65]
                        for g in range(4):
                            nc.tensor.matmul(
                                outT[0:65, g * 512:(g + 1) * 512],
                                vsl, w_ts[ck][:, g * 512:(g + 1) * 512],
                                start=(ck == 0), stop=(ck == N_SK - 1))

                # ---- postprocess head hh ----
                outS = postp.tile([65, S], f32, tag="outS", name=f"outS{hh}")
                nc.scalar.copy(outS[:], outT[0:65, :])
                denT = ps_sc.tile([128, 16], f32, tag="sc", name=f"denT{hh}")
                for ck in range(N_SK):
                    nc.tensor.transpose(
                        denT[:, ck:ck + 1],
                        outS[64:65, ck * 128:(ck + 1) * 128],
                        ident[64:65, 64:65])
                recT = postp.tile([128, 16], f32, tag="recT", name=f"recT{hh}")
                nc.vector.reciprocal_approx_fast(recT[:], denT[:])

                for ck in range(N_SK):
                    if hh == 0:
                        final_tiles[ck] = finp.tile(
                            [128, 128], f32, tag=f"fin{ck}", name=f"fin{ck}")
                    ft = final_tiles[ck]
                    ptf = ps_sc.tile([128, 64], f32, tag="sc",
                                     name=f"ftr{hh}_{ck}")
                    nc.tensor.transpose(
                        ptf[:], outS[0:64, ck * 128:(ck + 1) * 128],
                        ident[0:64, 0:64])
                    src = ptf[:].rearrange("p (t c) -> p t c", t=2)
                    dst = ft[:, hh * 64:(hh + 1) * 64].rearrange(
                        "p (c t) -> p t c", t=2)
                    nc.vector.tensor_scalar(
                        dst, src, recT[:, ck:ck + 1], None, ALU.mult)
                    if hh == 1:
                        nc.sync.dma_start(
                            OUT[ck * 128:(ck + 1) * 128, :], ft[:])

    nc.compile()
    return nc


# --------------------------------------------------------------------------
# host-side input prep
# --------------------------------------------------------------------------

def _prep_inputs(Q, V, K, Wq, bq, Wk, bk, Wv, bv):
    def xt(x):
        return np.ascontiguousarray(
            np.asarray(x).reshape(S, KCH).T).astype(np.float16)

    perm = np.zeros((128, 128), np.float32)
    for _hh in range(2):
        r0 = _hh * 64
        for j in range(32):
            perm[r0 + j, r0 + 32 + j] = -1.0
            perm[r0 + 32 + j, r0 + j] = 1.0
    shared = {
        "XTq": xt(np.asarray(Q)[0]),
        "XTk": xt(np.asarray(K)[0]),
        "XTv": xt(np.asarray(V)[0]),
        "IDENT": np.eye(128, dtype=np.float32),
        "PERM": np.ascontiguousarray(perm.T).astype(np.float16),
    }

    # full-model lhsT blocks [1024 x 1024]; col = g*64 + type*32 + j,
    # o = g*32 + j; rows 2i (real in-ch), 2i+1 (imag in-ch)
    colsA = (np.arange(NH)[:, None] * 64 + np.arange(32)).ravel()
    colsB = colsA + 32
    osA = (np.arange(NH)[:, None] * 32 + np.arange(32)).ravel()

    def build(W, b, variant):
        W = np.asarray(W, dtype=np.float32)
        b = np.asarray(b, dtype=np.float32)
        Wr_t, Wi_t = W[:, :, 0].T, W[:, :, 1].T      # [i, o]
        lhsT = np.zeros((KCH, KCH), np.float32)
        biasv = np.zeros(KCH, np.float32)
        if variant == "std":
            lhsT[0::2][:, colsA] = Wr_t[:, osA]
            lhsT[1::2][:, colsA] = -Wi_t[:, osA]
            lhsT[0::2][:, colsB] = Wi_t[:, osA]
            lhsT[1::2][:, colsB] = Wr_t[:, osA]
            biasv[colsA] = b[osA, 0]
            biasv[colsB] = b[osA, 1]
        else:  # kp: rows [-ki ; kr]
            lhsT[0::2][:, colsA] = -Wi_t[:, osA]
            lhsT[1::2][:, colsA] = -Wr_t[:, osA]
            lhsT[0::2][:, colsB] = Wr_t[:, osA]
            lhsT[1::2][:, colsB] = -Wi_t[:, osA]
            biasv[colsA] = -b[osA, 1]
            biasv[colsB] = b[osA, 0]
        return lhsT, biasv

    lq, bq_v = build(Wq, bq, "std")
    lq *= SCALE
    bq_v = bq_v * SCALE
    lk, bk_v = build(Wk, bk, "std")
    lkp, bkp_v = build(Wk, bk, "kp")
    lv, bv_v = build(Wv, bv, "std")

    def chunked(blk):  # [1024, 128] -> [128, 8*128] chunk-major
        return np.ascontiguousarray(
            blk.reshape(8, 128, 128).transpose(1, 0, 2).reshape(128, KCH)
        ).astype(np.float16)

    in_maps = []
    for c in range(N_CORES):
        sl = slice(c * 128, (c + 1) * 128)
        m = dict(shared)
        m["WBq"] = chunked(lq[:, sl])
        m["WBk"] = chunked(lk[:, sl])
        m["WBv"] = chunked(lv[:, sl])
        m["BIAS"] = np.stack(
            [bq_v[sl], bk_v[sl], bkp_v[sl], bv_v[sl]], axis=1).astype(
                np.float32).copy()
        in_maps.append(m)
    return in_maps


# --------------------------------------------------------------------------
# entry point
# --------------------------------------------------------------------------

def _get_program():
    global _PREPARED
    if _PREPARED is None:
        _PREPARED = _build_program()
    return _PREPARED


def kernel(**inputs):
    from concourse.bass_utils import run_bass_kernel_spmd
    nc = _get_program()
    in_maps = _prep_inputs(**inputs)
    res = run_bass_kernel_spmd(nc, in_maps, list(range(N_CORES)), trace=False)
    parts = [res.results[c]["OUT"] for c in range(N_CORES)]
    full = np.concatenate(parts, axis=1)          # [2048, 1024]
    return full.reshape(1, S, C, 2).astype(np.float32, copy=False)


# revision 14
# speedup vs baseline: 1.5334x; 1.0549x over previous
"""ComplexAttention Trainium2 kernel — 8-core SPMD, head-parallel sharding.

Self-contained: kernel(**inputs) takes the FULL inputs (as in
reference.setup_inputs()) and returns the FULL [1,2048,512,2] output.

Per core c (heads 2c, 2c+1):
  - 4 projections (q, k, k' = [-ki;kr] for imag scores, v) as fp32r matmuls,
    contraction over 1024 interleaved real input channels, K=128-chunk PSUM
    accumulation. Outputs are channel-major [128ch, 2048seq] tiles.
  - per head, per sk-chunk (16x128): transposed score tiles
    sT_r/sT_i [sk=128, sq=2048] via single-shot K=64 fp32r matmuls
    (lhsT = kT/kTp chunk, rhs = qT; 1/sqrt(32) folded into Wq host-side).
  - softmax without max-subtraction (max|s| ~ 19 << 88):
    z = re^2+im^2 (custom DVE ops), |s| = exp(0.5*ln z), w = exp(|s|)
    (ACT, single resident natural_log_exp_and_others table set).
  - AV: outT[65, 2048] += v_chunk[128sk, 65].T @ w[128sk, 2048sq] with a
    ones column producing softmax denominators in row 64. fp32r K=128 accum.
  - normalize + (real,imag) interleave folded into the final PE transpose
    back to [sq, ch] layout; per-chunk [128, 128] stores.
"""
import os
import sys
import types

import numpy as np

for _p in ("/opt/trn_rl_repo", "/root/.axon_site/_ro/trn_rl_repo"):
    if _p not in sys.path and os.path.isdir(_p):
        sys.path.append(_p)

N_CORES = 8
S = 2048
C = 512
NH = 16
HD = 32
KCH = 1024          # interleaved real input channels = 2*C
N_SK = S // 128     # 16 sk chunks
SCALE = 1.0 / np.sqrt(np.float32(HD))

SQRT_MODE = os.environ.get("CA_SQRT_MODE", "sqrt")   # "sqrt" | "lnexp"
W_SHIFT = 14.0      # w = exp(|s| - W_SHIFT); constant shift cancels in softmax
Z_FP16 = os.environ.get("CA_Z_FP16", "1") == "1"

_PREPARED = None    # compiled program cache across kernel() calls


# --------------------------------------------------------------------------
# framework tweaks (in-process only)
# --------------------------------------------------------------------------

def _patch_act_tables(keep=("sqrt_and_others", "exp_and_others",
                            "natural_log_exp_and_others")):
    """Restrict ACT table-set choice to a known set list so the inserter
    never picks a set that forces extra table loads."""
    import concourse.hw_specs as hw_specs
    import concourse.bacc as bacc_mod
    orig = hw_specs.get_activation_tables
    if getattr(bacc_mod.get_activation_tables, "_ca_patched", False):
        return

    def patched(arch):
        t = orig(arch)
        return {name: (funcs if name in keep else set())
                for name, funcs in t.items()}
    patched._ca_patched = True
    bacc_mod.get_activation_tables = patched


def _register_custom_ops():
    """Register SQ (x^2 from PSUM) and SQ_PLUS (x^2 + y) custom DVE ops."""
    import concourse.dve_ops as dmod
    from concourse.dve_ops import DveOp
    from concourse.dve_spec import Spec, Src0, Src1, sq, lower
    from concourse.dve_uop import DveOpSpec

    def reg(name, spec):
        if name in dmod._SUB_OPCODE_FOR_NAME:
            return next(o for o in dmod.OPS if o.name == name)
        row = dmod._CUSTOM_DVE_ROW_BASE + len(dmod.OPS)
        dmod._SUB_OPCODE_FOR_NAME[name] = row
        shas = {}
        from concourse.dve_spec import _has_src1
        for ver in ("v3", "v4"):
            tmp = DveOpSpec(name=name, opcode=row, uops=lower(spec, ver=ver),
                            rd1_en=_has_src1(spec))
            shas[ver] = tmp.sha(ver)
        op = DveOp(name, spec, subdim=False, uops_sha=shas)
        dmod.OPS.append(op)
        dmod.CUSTOM_DVE_SPECS[name] = op.spec
        return op

    sq_op = reg("CA_SQ", Spec(
        body=sq(Src0),
        reference=lambda in0, in1, s0, s1, imm2:
            (in0.astype(np.float32) ** 2).astype(np.float32)))
    sq_plus = reg("CA_SQ_PLUS", Spec(
        body=sq(Src0) + Src1,
        reference=lambda in0, in1, s0, s1, imm2:
            (in0.astype(np.float32) ** 2 + in1).astype(np.float32)))
    return sq_op, sq_plus


# --------------------------------------------------------------------------
# device program
# --------------------------------------------------------------------------

def _build_program():
    keep = (("sqrt_and_others", "exp_and_others") if SQRT_MODE == "sqrt"
            else ("natural_log_exp_and_others",))
    _patch_act_tables(keep)
    SQ, SQ_PLUS = _register_custom_ops()

    import concourse.tile as tile
    from concourse.tile_rust import add_dep_helper
    from concourse import bacc, mybir

    f32 = mybir.dt.float32
    f16 = mybir.dt.float16
    f32r = mybir.dt.float32r
    AF = mybir.ActivationFunctionType
    ALU = mybir.AluOpType
    zdt = f16 if Z_FP16 else f32

    nc = bacc.Bacc("TRN2", target_bir_lowering=False, debug=False,
                   num_devices=N_CORES)

    XT = {p: nc.dram_tensor(f"XT{p}", [KCH, S], f16, kind="ExternalInput").ap()
          for p in ("q", "k", "v")}
    WB = {p: nc.dram_tensor(f"WB{p}", [128, KCH], f16, kind="ExternalInput").ap()
          for p in ("q", "k", "v")}
    BIAS = nc.dram_tensor("BIAS", [128, 4], f32, kind="ExternalInput").ap()
    IDENT = nc.dram_tensor("IDENT", [128, 128], f32, kind="ExternalInput").ap()
    PERM = nc.dram_tensor("PERM", [128, 128], f16, kind="ExternalInput").ap()
    OUT = nc.dram_tensor("OUT", [S, 128], f32, kind="ExternalOutput").ap()

    with tile.TileContext(nc) as tc, \
         tc.tile_pool(name="persist", bufs=1) as persist, \
         tc.tile_pool(name="fin", bufs=1) as finp:
        with (
            tc.tile_pool(name="xstage", bufs=4) as xstage,
            tc.tile_pool(name="ps_proj", bufs=2, space="PSUM") as ps_proj,
        ):
            zeros1 = persist.tile([128, 1], f32)
            nc.gpsimd.memset(zeros1[:], 0.0)
            nc.const_aps.aps[(f32, 0.0)] = zeros1[:]
            eps1 = persist.tile([128, 1], f32)
            nc.gpsimd.memset(eps1[:], 1e-35)
            nc.const_aps.aps[(f32, 1e-35)] = eps1[:]
            wsh = persist.tile([128, 1], f32)
            nc.gpsimd.memset(wsh[:], -W_SHIFT)
            nc.const_aps.aps[(f32, -W_SHIFT)] = wsh[:]

            ident = persist.tile([128, 128], f32)
            nc.sync.dma_start(ident[:], IDENT[:])
            bias = persist.tile([128, 4], f32)
            nc.sync.dma_start(bias[:], BIAS[:])
            permT = persist.tile([128, 128], f16)
            nc.sync.dma_start(permT[:], PERM[:])

            wb = {}
            for p in ("q", "k", "v"):
                wb[p] = persist.tile([128, KCH], f16, tag=f"wb{p}",
                                     name=f"wb{p}")
                nc.sync.dma_start(wb[p][:], WB[p][:])

            # ---- phase A: projections, order v -> q -> k ----
            proj = {}
            bias_col = {"q": 0, "k": 1, "v": 3}
            for p in ("v", "q", "k"):
                psum_p = ps_proj.tile([128, S], f32, tag="proj",
                                      name=f"psum_{p}")
                for kc in range(8):
                    xch = xstage.tile([128, S], f16, tag="xch")
                    nc.sync.dma_start(xch[:], XT[p][kc * 128:(kc + 1) * 128, :])
                    for g in range(4):
                        nc.tensor.matmul(
                            psum_p[:, g * 512:(g + 1) * 512],
                            wb[p][:, kc * 128:(kc + 1) * 128],
                            xch[:, g * 512:(g + 1) * 512],
                            start=(kc == 0), stop=(kc == 7))
                dt_out = f32 if p == "v" else f16
                pt = persist.tile([128, S], dt_out, tag=f"proj{p}",
                                  name=f"proj{p}")
                nc.scalar.activation(pt[:], psum_p[:], AF.Identity,
                                     bias=bias[:, bias_col[p]:bias_col[p] + 1])
                proj[p] = pt

                if p == "v":
                    # v chunks (transposed) + ones columns, reuse proj slots
                    v_ch = []
                    for ck in range(N_SK):
                        vc = persist.tile([128, 132], f16, tag=f"vch{ck}",
                                          name=f"vch{ck}")
                        ptr = ps_proj.tile([128, 128], f32, tag="proj",
                                           name=f"vtr{ck}")
                        nc.tensor.transpose(
                            ptr[:], pt[:, ck * 128:(ck + 1) * 128], ident[:])
                        dst = vc[:].rearrange(
                            "p (h j) -> p h j", h=2)[:, :, 0:64]
                        srcv = ptr[:].rearrange("p (h j) -> p h j", h=2)
                        nc.vector.tensor_copy(dst, srcv)
                        ones_cols = vc[:].rearrange(
                            "p (h j) -> p h j", h=2)[:, :, 64:65]
                        nc.gpsimd.memset(ones_cols, 1.0)
                        v_ch.append(vc)

            # kTp = Perm @ kT (signed block-swap of the biased kT rows):
            # row hh*64+j (j<32) = -kT[hh*64+32+j]; row hh*64+32+j = kT[hh*64+j]
            kTp_t = persist.tile([128, S], f16, tag="projkp", name="projkp")
            for g in range(4):
                pp = ps_proj.tile([128, 512], f32, tag="proj",
                                  name=f"kppsum{g}")
                nc.tensor.matmul(pp[:], permT[:],
                                 proj["k"][:, g * 512:(g + 1) * 512],
                                 start=True, stop=True)
                nc.scalar.activation(kTp_t[:, g * 512:(g + 1) * 512], pp[:],
                                     AF.Identity)
            proj["kp"] = kTp_t

        # ---- zero-padded per-head rhs: K=128 matmuls keep the PE clock warm
        # (K<128 matmuls never register HAM activity -> 1.2GHz forever)
        qz = []
        for hh in range(2):
            qzt = persist.tile([128, S], f16, tag=f"qz{hh}", name=f"qz{hh}")
            r0 = hh * 64
            nc.vector.tensor_copy(qzt[r0:r0 + 64, :],
                                  proj["q"][r0:r0 + 64, :])
            nc.gpsimd.memset(qzt[64 - r0:128 - r0, :], 0.0)
            qz.append(qzt)

        # ---- phase B ----
        with (
            tc.tile_pool(name="ps_sc", bufs=2, space="PSUM") as ps_sc,
            tc.tile_pool(name="ps_out", bufs=1, space="PSUM") as ps_out,
            tc.tile_pool(name="zpool", bufs=6) as zpool,
            tc.tile_pool(name="spool", bufs=5) as spool,
            tc.tile_pool(name="wpool", bufs=6) as wpool,
            tc.tile_pool(name="tq", bufs=2) as tq,
            tc.tile_pool(name="postp", bufs=2) as postp,
        ):
            final_tiles = [None] * N_SK
            NB = 4                       # cks per ACT table batch

            for hh in range(2):
                qT = qz[hh]
                kT = proj["k"]
                kTp = proj["kp"]
                outT = ps_out.tile([65, S], f32, tag="outT", name=f"outT{hh}")

                prev_act_last = None
                for b in range(N_SK // NB):
                    cks = range(b * NB, (b + 1) * NB)
                    z_ts, s_ts, w_ts = {}, {}, {}
                    sq_insts, ex_insts = [], []
                    for ck in cks:
                        ksl = slice(ck * 128, (ck + 1) * 128)
                        z_t = zpool.tile([128, S], zdt, tag="z",
                                         name=f"z{hh}_{ck}")
                        z_ts[ck] = z_t
                        for r2 in range(2):
                            a_t = ps_sc.tile([128, 1024], f32, tag="sc",
                                             name=f"sa{hh}_{ck}_{r2}")
                            b_t = ps_sc.tile([128, 1024], f32, tag="sc",
                                             name=f"sb{hh}_{ck}_{r2}")
                            for gg in range(2):
                                g0 = r2 * 1024 + gg * 512
                                nc.tensor.matmul(
                                    a_t[:, gg * 512:(gg + 1) * 512],
                                    kT[:, ksl], qT[:, g0:g0 + 512],
                                    start=True, stop=True)
                            for gg in range(2):
                                g0 = r2 * 1024 + gg * 512
                                nc.tensor.matmul(
                                    b_t[:, gg * 512:(gg + 1) * 512],
                                    kTp[:, ksl], qT[:, g0:g0 + 512],
                                    start=True, stop=True)
                            t_t = tq.tile([128, 1024], f32, tag="t")
                            nc.vector._custom_dve(SQ, out=t_t[:], in0=a_t[:])
                            nc.vector._custom_dve(
                                SQ_PLUS,
                                out=z_t[:, r2 * 1024:(r2 + 1) * 1024],
                                in0=b_t[:], in1=t_t[:])
                    for ck in cks:
                        s_t = spool.tile([128, S], f32, tag="s",
                                         name=f"s{hh}_{ck}")
                        s_ts[ck] = s_t
                        if SQRT_MODE == "sqrt":
                            si = nc.scalar.activation(s_t[:], z_ts[ck][:],
                                                      AF.Sqrt)
                        else:
                            si = nc.scalar.activation(s_t[:], z_ts[ck][:],
                                                      AF.Ln, bias=1e-35)
                        sq_insts.append(si)
                    for ck in cks:
                        w_t = wpool.tile([128, S], f16, tag="w",
                                         name=f"w{hh}_{ck}")
                        w_ts[ck] = w_t
                        if SQRT_MODE == "sqrt":
                            ei = nc.scalar.activation(w_t[:], s_ts[ck][:],
                                                      AF.Exp, bias=-W_SHIFT)
                            ex_insts.append(ei)
                        else:
                            u2 = spool.tile([128, S], f32, tag="s",
                                            name=f"u2{hh}_{ck}")
                            nc.scalar.activation(u2[:], s_ts[ck][:], AF.Exp,
                                                 scale=0.5)
                            nc.scalar.activation(w_t[:], u2[:], AF.Exp,
                                                 bias=-W_SHIFT)
                    # pin ACT order: all sqrts, then all exps, per batch
                    if sq_insts and ex_insts:
                        if prev_act_last is not None:
                            add_dep_helper(prev_act_last.ins, sq_insts[0].ins,
                                           sync=False, reason="act batch")
                        add_dep_helper(sq_insts[-1].ins, ex_insts[0].ins, sync=False,
                                       reason="act batch")
                        prev_act_last = ex_insts[-1]
                    for ck in cks:
                        vsl = v_ch[ck][:, hh * 66:hh * 66 + 65]
                        for g in range(4):
                            nc.tensor.matmul(
                                outT[0:65, g * 512:(g + 1) * 512],
                                vsl, w_ts[ck][:, g * 512:(g + 1) * 512],
                                start=(ck == 0), stop=(ck == N_SK - 1))

                # ---- postprocess head hh ----
                outS = postp.tile([65, S], f32, tag="outS", name=f"outS{hh}")
                nc.scalar.copy(outS[:], outT[0:65, :])
                denT = ps_sc.tile([128, 16], f32, tag="sc", name=f"denT{hh}")
                for ck in range(N_SK):
                    nc.tensor.transpose(
                        denT[:, ck:ck + 1],
                        outS[64:65, ck * 128:(ck + 1) * 128],
                        ident[64:65, 64:65])
                recT = postp.tile([128, 16], f32, tag="recT", name=f"recT{hh}")
                nc.vector.reciprocal_approx_fast(recT[:], denT[:])

                for ck in range(N_SK):
                    if hh == 0:
                        final_tiles[ck] = finp.tile(
                            [128, 128], f32, tag=f"fin{ck}", name=f"fin{ck}")
                    ft = final_tiles[ck]
                    ptf = ps_sc.tile([128, 64], f32, tag="sc",
                                     name=f"ftr{hh}_{ck}")
                    nc.tensor.transpose(
                        ptf[:], outS[0:64, ck * 128:(ck + 1) * 128],
                        ident[0:64, 0:64])
                    src = ptf[:].rearrange("p (t c) -> p t c", t=2)
                    dst = ft[:, hh * 64:(hh + 1) * 64].rearrange(
                        "p (c t) -> p t c", t=2)
                    nc.vector.tensor_scalar(
                        dst, src, recT[:, ck:ck + 1], None, ALU.mult)
                    if hh == 1:
                        nc.sync.dma_start(
                            OUT[ck * 128:(ck + 1) * 128, :], ft[:])

    nc.compile()
    return nc


# --------------------------------------------------------------------------
# host-side input prep
# --------------------------------------------------------------------------

def _prep_inputs(Q, V, K, Wq, bq, Wk, bk, Wv, bv):
    def xt(x):
        return np.ascontiguousarray(
            np.asarray(x).reshape(S, KCH).T).astype(np.float16)

    perm = np.zeros((128, 128), np.float32)
    for _hh in range(2):
        r0 = _hh * 64
        for j in range(32):
            perm[r0 + j, r0 + 32 + j] = -1.0
            perm[r0 + 32 + j, r0 + j] = 1.0
    shared = {
        "XTq": xt(np.asarray(Q)[0]),
        "XTk": xt(np.asarray(K)[0]),
        "XTv": xt(np.asarray(V)[0]),
        "IDENT": np.eye(128, dtype=np.float32),
        "PERM": np.ascontiguousarray(perm.T).astype(np.float16),
    }

    # full-model lhsT blocks [1024 x 1024]; col = g*64 + type*32 + j,
    # o = g*32 + j; rows 2i (real in-ch), 2i+1 (imag in-ch)
    colsA = (np.arange(NH)[:, None] * 64 + np.arange(32)).ravel()
    colsB = colsA + 32
    osA = (np.arange(NH)[:, None] * 32 + np.arange(32)).ravel()

    def build(W, b, variant):
        W = np.asarray(W, dtype=np.float32)
        b = np.asarray(b, dtype=np.float32)
        Wr_t, Wi_t = W[:, :, 0].T, W[:, :, 1].T      # [i, o]
        lhsT = np.zeros((KCH, KCH), np.float32)
        biasv = np.zeros(KCH, np.float32)
        if variant == "std":
            lhsT[0::2][:, colsA] = Wr_t[:, osA]
            lhsT[1::2][:, colsA] = -Wi_t[:, osA]
            lhsT[0::2][:, colsB] = Wi_t[:, osA]
            lhsT[1::2][:, colsB] = Wr_t[:, osA]
            biasv[colsA] = b[osA, 0]
            biasv[colsB] = b[osA, 1]
        else:  # kp: rows [-ki ; kr]
            lhsT[0::2][:, colsA] = -Wi_t[:, osA]
            lhsT[1::2][:, colsA] = -Wr_t[:, osA]
            lhsT[0::2][:, colsB] = Wr_t[:, osA]
            lhsT[1::2][:, colsB] = -Wi_t[:, osA]
            biasv[colsA] = -b[osA, 1]
            biasv[colsB] = b[osA, 0]
        return lhsT, biasv

    lq, bq_v = build(Wq, bq, "std")
    lq *= SCALE
    bq_v = bq_v * SCALE
    lk, bk_v = build(Wk, bk, "std")
    lkp, bkp_v = build(Wk, bk, "kp")
    lv, bv_v = build(Wv, bv, "std")

    def chunked(blk):  # [1024, 128] -> [128, 8*128] chunk-major
        return np.ascontiguousarray(
            blk.reshape(8, 128, 128).transpose(1, 0, 2).reshape(128, KCH)
        ).astype(np.float16)

    in_maps = []
    for c in range(N_CORES):
        sl = slice(c * 128, (c + 1) * 128)
        m = dict(shared)
        m["WBq"] = chunked(lq[:, sl])
        m["WBk"] = chunked(lk[:, sl])
        m["WBv"] = chunked(lv[:, sl])
        m["BIAS"] = np.stack(
            [bq_v[sl], bk_v[sl], bkp_v[sl], bv_v[sl]], axis=1).astype(
                np.float32).copy()
        in_maps.append(m)
    return in_maps


# --------------------------------------------------------------------------
# entry point
# --------------------------------------------------------------------------

def _get_program():
    global _PREPARED
    if _PREPARED is None:
        _PREPARED = _build_program()
    return _PREPARED


def kernel(**inputs):
    from concourse.bass_utils import run_bass_kernel_spmd
    nc = _get_program()
    in_maps = _prep_inputs(**inputs)
    res = run_bass_kernel_spmd(nc, in_maps, list(range(N_CORES)), trace=False)
    parts = [res.results[c]["OUT"] for c in range(N_CORES)]
    full = np.concatenate(parts, axis=1)          # [2048, 1024]
    return full.reshape(1, S, C, 2).astype(np.float32, copy=False)


# revision 15
# speedup vs baseline: 1.6073x; 1.0482x over previous
"""ComplexAttention Trainium2 kernel — 8-core SPMD, head-parallel sharding.

Self-contained: kernel(**inputs) takes the FULL inputs (as in
reference.setup_inputs()) and returns the FULL [1,2048,512,2] output.

Per core c (heads 2c, 2c+1):
  - 4 projections (q, k, k' = [-ki;kr] for imag scores, v) as fp32r matmuls,
    contraction over 1024 interleaved real input channels, K=128-chunk PSUM
    accumulation. Outputs are channel-major [128ch, 2048seq] tiles.
  - per head, per sk-chunk (16x128): transposed score tiles
    sT_r/sT_i [sk=128, sq=2048] via single-shot K=64 fp32r matmuls
    (lhsT = kT/kTp chunk, rhs = qT; 1/sqrt(32) folded into Wq host-side).
  - softmax without max-subtraction (max|s| ~ 19 << 88):
    z = re^2+im^2 (custom DVE ops), |s| = exp(0.5*ln z), w = exp(|s|)
    (ACT, single resident natural_log_exp_and_others table set).
  - AV: outT[65, 2048] += v_chunk[128sk, 65].T @ w[128sk, 2048sq] with a
    ones column producing softmax denominators in row 64. fp32r K=128 accum.
  - normalize + (real,imag) interleave folded into the final PE transpose
    back to [sq, ch] layout; per-chunk [128, 128] stores.
"""
import os
import sys
import types

import numpy as np

for _p in ("/opt/trn_rl_repo", "/root/.axon_site/_ro/trn_rl_repo"):
    if _p not in sys.path and os.path.isdir(_p):
        sys.path.append(_p)

N_CORES = 8
S = 2048
C = 512
NH = 16
HD = 32
KCH = 1024          # interleaved real input channels = 2*C
N_SK = S // 128     # 16 sk chunks
SCALE = 1.0 / np.sqrt(np.float32(HD))

SQRT_MODE = os.environ.get("CA_SQRT_MODE", "sqrt")   # "sqrt" | "lnexp"
W_SHIFT = 14.0      # w = exp(|s| - W_SHIFT); constant shift cancels in softmax
Z_FP16 = os.environ.get("CA_Z_FP16", "1") == "1"

_PREPARED = None    # compiled program cache across kernel() calls


# --------------------------------------------------------------------------
# framework tweaks (in-process only)
# --------------------------------------------------------------------------

def _patch_act_tables(keep=("sqrt_and_others", "exp_and_others",
                            "natural_log_exp_and_others")):
    """Restrict ACT table-set choice to a known set list so the inserter
    never picks a set that forces extra table loads."""
    import concourse.hw_specs as hw_specs
    import concourse.bacc as bacc_mod
    orig = hw_specs.get_activation_tables
    if getattr(bacc_mod.get_activation_tables, "_ca_patched", False):
        return

    def patched(arch):
        t = orig(arch)
        return {name: (funcs if name in keep else set())
                for name, funcs in t.items()}
    patched._ca_patched = True
    bacc_mod.get_activation_tables = patched


def _register_custom_ops():
    """Register SQ (x^2 from PSUM) and SQ_PLUS (x^2 + y) custom DVE ops."""
    import concourse.dve_ops as dmod
    from concourse.dve_ops import DveOp
    from concourse.dve_spec import Spec, Src0, Src1, sq, lower
    from concourse.dve_uop import DveOpSpec

    def reg(name, spec):
        if name in dmod._SUB_OPCODE_FOR_NAME:
            return next(o for o in dmod.OPS if o.name == name)
        row = dmod._CUSTOM_DVE_ROW_BASE + len(dmod.OPS)
        dmod._SUB_OPCODE_FOR_NAME[name] = row
        shas = {}
        from concourse.dve_spec import _has_src1
        for ver in ("v3", "v4"):
            tmp = DveOpSpec(name=name, opcode=row, uops=lower(spec, ver=ver),
                            rd1_en=_has_src1(spec))
            shas[ver] = tmp.sha(ver)
        op = DveOp(name, spec, subdim=False, uops_sha=shas)
        dmod.OPS.append(op)
        dmod.CUSTOM_DVE_SPECS[name] = op.spec
        return op

    sq_op = reg("CA_SQ", Spec(
        body=sq(Src0),
        reference=lambda in0, in1, s0, s1, imm2:
            (in0.astype(np.float32) ** 2).astype(np.float32)))
    sq_plus = reg("CA_SQ_PLUS", Spec(
        body=sq(Src0) + Src1,
        reference=lambda in0, in1, s0, s1, imm2:
            (in0.astype(np.float32) ** 2 + in1).astype(np.float32)))
    return sq_op, sq_plus


# --------------------------------------------------------------------------
# device program
# --------------------------------------------------------------------------

def _build_program():
    keep = (("sqrt_and_others", "exp_and_others") if SQRT_MODE == "sqrt"
            else ("natural_log_exp_and_others",))
    _patch_act_tables(keep)
    SQ, SQ_PLUS = _register_custom_ops()

    import concourse.tile as tile
    from concourse.tile_rust import add_dep_helper
    from concourse import bacc, mybir

    f32 = mybir.dt.float32
    f16 = mybir.dt.float16
    f32r = mybir.dt.float32r
    AF = mybir.ActivationFunctionType
    ALU = mybir.AluOpType
    zdt = f16 if Z_FP16 else f32

    nc = bacc.Bacc("TRN2", target_bir_lowering=False, debug=False,
                   num_devices=N_CORES)

    XT = {p: nc.dram_tensor(f"XT{p}", [KCH, S], f16, kind="ExternalInput").ap()
          for p in ("q", "k", "v")}
    WB = {p: nc.dram_tensor(f"WB{p}", [128, KCH], f16, kind="ExternalInput").ap()
          for p in ("q", "k", "v")}
    BIAS = nc.dram_tensor("BIAS", [128, 4], f32, kind="ExternalInput").ap()
    IDENT = nc.dram_tensor("IDENT", [128, 128], f32, kind="ExternalInput").ap()
    PERM = nc.dram_tensor("PERM", [128, 128], f16, kind="ExternalInput").ap()
    OUT = nc.dram_tensor("OUT", [S, 128], f32, kind="ExternalOutput").ap()

    with tile.TileContext(nc) as tc, \
         tc.tile_pool(name="persist", bufs=1) as persist, \
         tc.tile_pool(name="fin", bufs=1) as finp:
        with (
            tc.tile_pool(name="xstage", bufs=4) as xstage,
            tc.tile_pool(name="ps_proj", bufs=2, space="PSUM") as ps_proj,
        ):
            zeros1 = persist.tile([128, 1], f32)
            nc.gpsimd.memset(zeros1[:], 0.0)
            nc.const_aps.aps[(f32, 0.0)] = zeros1[:]
            eps1 = persist.tile([128, 1], f32)
            nc.gpsimd.memset(eps1[:], 1e-35)
            nc.const_aps.aps[(f32, 1e-35)] = eps1[:]
            wsh = persist.tile([128, 1], f32)
            nc.gpsimd.memset(wsh[:], -W_SHIFT)
            nc.const_aps.aps[(f32, -W_SHIFT)] = wsh[:]

            ident = persist.tile([128, 128], f32)
            nc.sync.dma_start(ident[:], IDENT[:])
            bias = persist.tile([128, 4], f32)
            nc.sync.dma_start(bias[:], BIAS[:])
            permT = persist.tile([128, 128], f16)
            nc.sync.dma_start(permT[:], PERM[:])

            wb = {}
            for p in ("q", "k", "v"):
                wb[p] = persist.tile([128, KCH], f16, tag=f"wb{p}",
                                     name=f"wb{p}")
                nc.sync.dma_start(wb[p][:], WB[p][:])

            # ---- phase A: projections, order v -> q -> k ----
            proj = {}
            bias_col = {"q": 0, "k": 1, "v": 3}
            for p in ("v", "q", "k"):
                psum_p = ps_proj.tile([128, S], f32, tag="proj",
                                      name=f"psum_{p}")
                for kc in range(8):
                    xch = xstage.tile([128, S], f16, tag="xch")
                    nc.sync.dma_start(xch[:], XT[p][kc * 128:(kc + 1) * 128, :])
                    for g in range(4):
                        nc.tensor.matmul(
                            psum_p[:, g * 512:(g + 1) * 512],
                            wb[p][:, kc * 128:(kc + 1) * 128],
                            xch[:, g * 512:(g + 1) * 512],
                            start=(kc == 0), stop=(kc == 7))
                dt_out = f32 if p == "v" else f16
                pt = persist.tile([128, S], dt_out, tag=f"proj{p}",
                                  name=f"proj{p}")
                nc.scalar.activation(pt[:], psum_p[:], AF.Identity,
                                     bias=bias[:, bias_col[p]:bias_col[p] + 1])
                proj[p] = pt

                if p == "v":
                    # v chunks (transposed) + ones columns, reuse proj slots
                    v_ch = []
                    for ck in range(N_SK):
                        vc = persist.tile([128, 132], f16, tag=f"vch{ck}",
                                          name=f"vch{ck}")
                        ptr = ps_proj.tile([128, 128], f32, tag="proj",
                                           name=f"vtr{ck}")
                        nc.tensor.transpose(
                            ptr[:], pt[:, ck * 128:(ck + 1) * 128], ident[:])
                        dst = vc[:].rearrange(
                            "p (h j) -> p h j", h=2)[:, :, 0:64]
                        srcv = ptr[:].rearrange("p (h j) -> p h j", h=2)
                        nc.vector.tensor_copy(dst, srcv)
                        ones_cols = vc[:].rearrange(
                            "p (h j) -> p h j", h=2)[:, :, 64:65]
                        nc.gpsimd.memset(ones_cols, 1.0)
                        v_ch.append(vc)

            # kTp = Perm @ kT (signed block-swap of the biased kT rows):
            # row hh*64+j (j<32) = -kT[hh*64+32+j]; row hh*64+32+j = kT[hh*64+j]
            kTp_t = persist.tile([128, S], f16, tag="projkp", name="projkp")
            for g in range(4):
                pp = ps_proj.tile([128, 512], f32, tag="proj",
                                  name=f"kppsum{g}")
                nc.tensor.matmul(pp[:], permT[:],
                                 proj["k"][:, g * 512:(g + 1) * 512],
                                 start=True, stop=True)
                nc.scalar.activation(kTp_t[:, g * 512:(g + 1) * 512], pp[:],
                                     AF.Identity)
            proj["kp"] = kTp_t

        # ---- zero-padded per-head rhs: K=128 matmuls keep the PE clock warm
        # (K<128 matmuls never register HAM activity -> 1.2GHz forever)
        qz = []
        for hh in range(2):
            qzt = persist.tile([128, S], f16, tag=f"qz{hh}", name=f"qz{hh}")
            r0 = hh * 64
            nc.vector.tensor_copy(qzt[r0:r0 + 64, :],
                                  proj["q"][r0:r0 + 64, :])
            nc.gpsimd.memset(qzt[64 - r0:128 - r0, :], 0.0)
            qz.append(qzt)

        # ---- phase B ----
        with (
            tc.tile_pool(name="ps_sc", bufs=2, space="PSUM") as ps_sc,
            tc.tile_pool(name="ps_out", bufs=1, space="PSUM") as ps_out,
            tc.tile_pool(name="zpool", bufs=6) as zpool,
            tc.tile_pool(name="spool", bufs=5) as spool,
            tc.tile_pool(name="wpool", bufs=6) as wpool,
            tc.tile_pool(name="tq", bufs=2) as tq,
            tc.tile_pool(name="postp", bufs=2) as postp,
        ):
            final_tiles = [None] * N_SK
            NB = 4                       # cks per ACT table batch

            for hh in range(2):
                qT = qz[hh]
                kT = proj["k"]
                kTp = proj["kp"]
                outT = ps_out.tile([65, S], f32, tag="outT", name=f"outT{hh}")

                prev_act_last = None
                for b in range(N_SK // NB):
                    cks = range(b * NB, (b + 1) * NB)
                    z_ts, s_ts, w_ts = {}, {}, {}
                    sq_insts, ex_insts = [], []
                    for ck in cks:
                        ksl = slice(ck * 128, (ck + 1) * 128)
                        z_t = zpool.tile([128, S], zdt, tag="z",
                                         name=f"z{hh}_{ck}")
                        z_ts[ck] = z_t
                        for r2 in range(2):
                            a_t = ps_sc.tile([128, 1024], f32, tag="sc",
                                             name=f"sa{hh}_{ck}_{r2}")
                            b_t = ps_sc.tile([128, 1024], f32, tag="sc",
                                             name=f"sb{hh}_{ck}_{r2}")
                            for gg in range(2):
                                g0 = r2 * 1024 + gg * 512
                                nc.tensor.matmul(
                                    a_t[:, gg * 512:(gg + 1) * 512],
                                    kT[:, ksl], qT[:, g0:g0 + 512],
                                    start=True, stop=True)
                            for gg in range(2):
                                g0 = r2 * 1024 + gg * 512
                                nc.tensor.matmul(
                                    b_t[:, gg * 512:(gg + 1) * 512],
                                    kTp[:, ksl], qT[:, g0:g0 + 512],
                                    start=True, stop=True)
                            t_t = tq.tile([128, 1024], f32, tag="t")
                            nc.vector._custom_dve(SQ, out=t_t[:], in0=a_t[:])
                            nc.vector._custom_dve(
                                SQ_PLUS,
                                out=z_t[:, r2 * 1024:(r2 + 1) * 1024],
                                in0=b_t[:], in1=t_t[:])
                    for ck in cks:
                        s_t = spool.tile([128, S], f32, tag="s",
                                         name=f"s{hh}_{ck}")
                        s_ts[ck] = s_t
                        if SQRT_MODE == "sqrt":
                            si = nc.scalar.activation(s_t[:], z_ts[ck][:],
                                                      AF.Sqrt)
                        else:
                            si = nc.scalar.activation(s_t[:], z_ts[ck][:],
                                                      AF.Ln, bias=1e-35)
                        sq_insts.append(si)
                    for ck in cks:
                        w_t = wpool.tile([128, S], f16, tag="w",
                                         name=f"w{hh}_{ck}")
                        w_ts[ck] = w_t
                        if SQRT_MODE == "sqrt":
                            ei = nc.scalar.activation(w_t[:], s_ts[ck][:],
                                                      AF.Exp, bias=-W_SHIFT)
                            ex_insts.append(ei)
                        else:
                            u2 = spool.tile([128, S], f32, tag="s",
                                            name=f"u2{hh}_{ck}")
                            nc.scalar.activation(u2[:], s_ts[ck][:], AF.Exp,
                                                 scale=0.5)
                            nc.scalar.activation(w_t[:], u2[:], AF.Exp,
                                                 bias=-W_SHIFT)
                    # pin ACT order: all sqrts, then all exps, per batch
                    if sq_insts and ex_insts:
                        if prev_act_last is not None:
                            add_dep_helper(sq_insts[0].ins, prev_act_last.ins,
                                           sync=False, reason="act batch")
                        add_dep_helper(ex_insts[0].ins, sq_insts[-1].ins, sync=False,
                                       reason="act batch")
                        prev_act_last = ex_insts[-1]
                    for ck in cks:
                        vsl = v_ch[ck][:, hh * 66:hh * 66 + 65]
                        for g in range(4):
                            nc.tensor.matmul(
                                outT[0:65, g * 512:(g + 1) * 512],
                                vsl, w_ts[ck][:, g * 512:(g + 1) * 512],
                                start=(ck == 0), stop=(ck == N_SK - 1))

                # ---- postprocess head hh ----
                outS = postp.tile([65, S], f32, tag="outS", name=f"outS{hh}")
                nc.scalar.copy(outS[:], outT[0:65, :])
                denT = ps_sc.tile([128, 16], f32, tag="sc", name=f"denT{hh}")
                for ck in range(N_SK):
                    nc.tensor.transpose(
                        denT[:, ck:ck + 1],
                        outS[64:65, ck * 128:(ck + 1) * 128],
                        ident[64:65, 64:65])
                recT = postp.tile([128, 16], f32, tag="recT", name=f"recT{hh}")
                nc.vector.reciprocal_approx_fast(recT[:], denT[:])

                for ck in range(N_SK):
                    if hh == 0:
                        final_tiles[ck] = finp.tile(
                            [128, 128], f32, tag=f"fin{ck}", name=f"fin{ck}")
                    ft = final_tiles[ck]
                    ptf = ps_sc.tile([128, 64], f32, tag="sc",
                                     name=f"ftr{hh}_{ck}")
                    nc.tensor.transpose(
                        ptf[:], outS[0:64, ck * 128:(ck + 1) * 128],
                        ident[0:64, 0:64])
                    src = ptf[:].rearrange("p (t c) -> p t c", t=2)
                    dst = ft[:, hh * 64:(hh + 1) * 64].rearrange(
                        "p (c t) -> p t c", t=2)
                    nc.vector.tensor_scalar(
                        dst, src, recT[:, ck:ck + 1], None, ALU.mult)
                    if hh == 1:
                        nc.sync.dma_start(
                            OUT[ck * 128:(ck + 1) * 128, :], ft[:])

    nc.compile()
    return nc


# --------------------------------------------------------------------------
# host-side input prep
# --------------------------------------------------------------------------

def _prep_inputs(Q, V, K, Wq, bq, Wk, bk, Wv, bv):
    def xt(x):
        return np.ascontiguousarray(
            np.asarray(x).reshape(S, KCH).T).astype(np.float16)

    perm = np.zeros((128, 128), np.float32)
    for _hh in range(2):
        r0 = _hh * 64
        for j in range(32):
            perm[r0 + j, r0 + 32 + j] = -1.0
            perm[r0 + 32 + j, r0 + j] = 1.0
    shared = {
        "XTq": xt(np.asarray(Q)[0]),
        "XTk": xt(np.asarray(K)[0]),
        "XTv": xt(np.asarray(V)[0]),
        "IDENT": np.eye(128, dtype=np.float32),
        "PERM": np.ascontiguousarray(perm.T).astype(np.float16),
    }

    # full-model lhsT blocks [1024 x 1024]; col = g*64 + type*32 + j,
    # o = g*32 + j; rows 2i (real in-ch), 2i+1 (imag in-ch)
    colsA = (np.arange(NH)[:, None] * 64 + np.arange(32)).ravel()
    colsB = colsA + 32
    osA = (np.arange(NH)[:, None] * 32 + np.arange(32)).ravel()

    def build(W, b, variant):
        W = np.asarray(W, dtype=np.float32)
        b = np.asarray(b, dtype=np.float32)
        Wr_t, Wi_t = W[:, :, 0].T, W[:, :, 1].T      # [i, o]
        lhsT = np.zeros((KCH, KCH), np.float32)
        biasv = np.zeros(KCH, np.float32)
        if variant == "std":
            lhsT[0::2][:, colsA] = Wr_t[:, osA]
            lhsT[1::2][:, colsA] = -Wi_t[:, osA]
            lhsT[0::2][:, colsB] = Wi_t[:, osA]
            lhsT[1::2][:, colsB] = Wr_t[:, osA]
            biasv[colsA] = b[osA, 0]
            biasv[colsB] = b[osA, 1]
        else:  # kp: rows [-ki ; kr]
            lhsT[0::2][:, colsA] = -Wi_t[:, osA]
            lhsT[1::2][:, colsA] = -Wr_t[:, osA]
            lhsT[0::2][:, colsB] = Wr_t[:, osA]
            lhsT[1::2][:, colsB] = -Wi_t[:, osA]
            biasv[colsA] = -b[osA, 1]
            biasv[colsB] = b[osA, 0]
        return lhsT, biasv

    lq, bq_v = build(Wq, bq, "std")
    lq *= SCALE
    bq_v = bq_v * SCALE
    lk, bk_v = build(Wk, bk, "std")
    lkp, bkp_v = build(Wk, bk, "kp")
    lv, bv_v = build(Wv, bv, "std")

    def chunked(blk):  # [1024, 128] -> [128, 8*128] chunk-major
        return np.ascontiguousarray(
            blk.reshape(8, 128, 128).transpose(1, 0, 2).reshape(128, KCH)
        ).astype(np.float16)

    in_maps = []
    for c in range(N_CORES):
        sl = slice(c * 128, (c + 1) * 128)
        m = dict(shared)
        m["WBq"] = chunked(lq[:, sl])
        m["WBk"] = chunked(lk[:, sl])
        m["WBv"] = chunked(lv[:, sl])
        m["BIAS"] = np.stack(
            [bq_v[sl], bk_v[sl], bkp_v[sl], bv_v[sl]], axis=1).astype(
                np.float32).copy()
        in_maps.append(m)
    return in_maps


# --------------------------------------------------------------------------
# entry point
# --------------------------------------------------------------------------

def _get_program():
    global _PREPARED
    if _PREPARED is None:
        _PREPARED = _build_program()
    return _PREPARED


def kernel(**inputs):
    from concourse.bass_utils import run_bass_kernel_spmd
    nc = _get_program()
    in_maps = _prep_inputs(**inputs)
    res = run_bass_kernel_spmd(nc, in_maps, list(range(N_CORES)), trace=False)
    parts = [res.results[c]["OUT"] for c in range(N_CORES)]
    full = np.concatenate(parts, axis=1)          # [2048, 1024]
    return full.reshape(1, S, C, 2).astype(np.float32, copy=False)
